# revision 5
# baseline (speedup 1.0000x reference)
"""CRF NLL loss kernel v2 for Trainium2 (Bass/Tile).

B=4096, L=4096, T=2, mask all-ones.  8 cores, data-parallel over batch.

Algorithm (per core, 512 seqs = 4 groups of 128 partitions):
  * Host: fold start_transitions into position-0 emissions; bf16 planar
    layout with pairs bit-reversed within 128-position blocks:
    planes (Ehi0, Ehi1, Elo0, Elo1) each [P, 2048]; tag planes (thi, tlo)
    bf16; tag-only gold terms computed on host.
  * Device per group:
      w-sums  W[(i,b)] = Ehi_i + Elo_b            (1 bf16 TT, 2x mode)
      gold: D = e1-e0 (TT), tD = t*D (pool STT, in place over tags),
            4x-mode TS accumulates
      6 exp streams with transition consts folded into ACT bias
      pair matrices P_ij = G0_ij w_i0 + G1_ij w_i1 (1 TS + 2 TT)
      pair-0 patched to alpha-init leaf (3 tiny ops)
      linear tree: 6 levels of constant-free 2x2 products (3 TT each)
      Ln -> f32 block logs into a persistent per-group buffer
  * Post-loop: log-domain top tree batched across groups (pool + ACT),
    final assembly, one DMA out.
  * Device outputs per seq: lT00, lT10 (log alpha), sum_e0, sum_tD.
    Host: logZ = CB*L + lse(en + lT); gold = sums + host tag part; mean.
"""

import numpy as np
import ml_dtypes
from contextlib import ExitStack

import concourse.bass as bass
import concourse.tile as tile
from concourse import mybir
from concourse.bass_utils import run_bass_kernel_spmd

AF = mybir.ActivationFunctionType
OP = mybir.AluOpType
F32 = mybir.dt.float32
BF16 = mybir.dt.bfloat16
NPBF = ml_dtypes.bfloat16

N_CORES = 8
P = 128            # SBUF partitions
G = 4              # groups of 128 seqs per core
L = 4096
NBLK = 32          # 128-position blocks per sequence
BLKP = 64          # pairs per block
PS = L // 2        # pairs per group-row = plane size (2048)


def _ap(t, off, dims):
    base = t[:]
    return bass.AP(tensor=base.tensor, offset=base.offset + off,
                   ap=[base.ap[0]] + [list(d) for d in dims])


def _split_multiwaits(nc):
    """Walrus here accepts only one sem wait per instruction; hoist extras
    onto same-engine single-wait drains."""
    for f in nc.m.functions:
        for b in f.blocks:
            out = []
            changed = False
            for ins in b.instructions:
                si = ins.sync_info
                if si is not None and si.on_wait and len(si.on_wait) > 1:
                    waits = list(si.on_wait)
                    for k, w in enumerate(waits[:-1]):
                        d = mybir.InstDrain(name=f"{ins.name}-wsplit{k}")
                        d.engine = ins.engine
                        d.sync_info = mybir.SyncInfo(on_wait=[w], on_update=[])
                        nc.register_instruction(d, overwrite=True)
                        out.append(d)
                    ins.sync_info = mybir.SyncInfo(
                        on_wait=[waits[-1]], on_update=list(si.on_update or []))
                    changed = True
                out.append(ins)
            if changed:
                b.instructions = out
    return nc


def _host_consts(transitions, CB):
    tr = np.asarray(transitions, np.float64)
    c = {}
    c["bh"] = tuple(float(tr[i, 0] + tr[0, 0] - 2 * CB) for i in (0, 1))
    c["bc"] = tuple(float(tr[i, 1] + tr[1, 0] - 2 * CB) for i in (0, 1))
    c["bd"] = tuple(float(tr[i, 1] + tr[1, 1] - 2 * CB) for i in (0, 1))
    c["delta"] = float(np.exp(tr[0, 1] - tr[0, 0]))
    c["p1"] = float(np.exp(tr[1, 0] - tr[0, 0]))   # K10/K00
    c["p2"] = float(np.exp(-tr[1, 0]))             # 1/K10
    c["CB"] = float(CB)
    return tuple(sorted(c.items()))


def _build(consts, repeat=1, ablate=()):
    c = dict(consts)
    nc = bass.Bass()
    em = nc.dram_tensor("emissions", [G * P, 4 * PS], BF16, kind="ExternalInput")
    tg = nc.dram_tensor("tagsf", [G * P, 2 * PS], BF16, kind="ExternalInput")
    outp = nc.dram_tensor("outp", [G * P, 4], F32, kind="ExternalOutput")

    with tile.TileContext(nc) as tc, ExitStack() as ctx:
        io = ctx.enter_context(tc.tile_pool(name="io", bufs=2))
        fr = ctx.enter_context(tc.tile_pool(name="fr", bufs=2))
        wk = ctx.enter_context(tc.tile_pool(name="wk", bufs=1))
        ps = ctx.enter_context(tc.tile_pool(name="ps", bufs=1))

        BIAS = ps.tile([P, 8], F32, tag="BIAS")
        bvals = [c["bh"][0], c["bh"][1], c["bc"][0], c["bc"][1],
                 c["bd"][0], c["bd"][1]]
        for k, bv in enumerate(bvals):
            nc.vector.memset(BIAS[:, k:k + 1], float(bv))
        # persistent: block logs for all groups, output accumulators
        LBA = ps.tile([P, 4 * G * NBLK], F32, tag="LBA")   # (i,j)-plane, g, blk
        ACA = ps.tile([P, 4 * G], F32, tag="ACA")

        for _rep in range(repeat):
            for g in range(G):
                EM = io.tile([P, 4 * PS], BF16, tag="EM")
                nc.sync.dma_start(out=EM, in_=em[g * P:(g + 1) * P, :])
                TG = io.tile([P, 2 * PS], BF16, tag="TG")
                nc.scalar.dma_start(out=TG, in_=tg[g * P:(g + 1) * P, :])

                # ---- w sums: W[(i,b)] = Ehi_i + Elo_b ----
                W = fr.tile([P, 4 * PS], BF16, tag="W")
                nc.vector.tensor_tensor(
                    out=_ap(W, 0, [[2 * PS, 2], [PS, 2], [1, PS]]),
                    in0=_ap(EM, 0, [[PS, 2], [0, 2], [1, PS]]),
                    in1=_ap(EM, 2 * PS, [[0, 2], [PS, 2], [1, PS]]),
                    op=OP.add)

                # ---- gold (early: fills DVE while ACT does exps) ----
                # PM is declared here; accumulate-op scratch outputs dump
                # into regions that later get fully overwritten (PM) or are
                # dead (D), so no extra tiles are needed.
                PM = wk.tile([P, 4 * PS], BF16, tag="PM")
                D = wk.tile([P, 2 * PS], BF16, tag="D")
                if "gold" in ablate:
                    nc.vector.memset(D, 0.5)
                if "gold" not in ablate:
                    nc.vector.tensor_tensor(
                        out=D[:, 0:PS],
                        in0=_ap(EM, PS, [[1, PS]]),
                        in1=_ap(EM, 0, [[1, PS]]), op=OP.subtract)
                    nc.gpsimd.tensor_tensor(
                        out=D[:, PS:2 * PS],
                        in0=_ap(EM, 3 * PS, [[1, PS]]),
                        in1=_ap(EM, 2 * PS, [[1, PS]]), op=OP.subtract)
                if "gold" not in ablate:
                    nc.scalar.activation(
                        _ap(PM, 0, [[PS, 2], [1, PS]]),
                        _ap(EM, 0, [[2 * PS, 2], [1, PS]]), AF.Copy,
                        accum_out=ACA[:, 4 * g + 2:4 * g + 3])
                    # tD in place over TG (pool)
                    nc.gpsimd.tensor_tensor(out=TG, in0=TG, in1=D, op=OP.mult)
                    nc.scalar.activation(
                        D, TG, AF.Copy,
                        accum_out=ACA[:, 4 * g + 3:4 * g + 4])

                # ---- exp streams: wh0,wh1,wc0,wc1,wd0,wd1 ----
                EX = fr.tile([P, 6 * PS], BF16, tag="EX")
                if "exp" in ablate:
                    nc.scalar.activation(EX[:, 0:PS], W[:, 0:PS], AF.Exp,
                                         bias=BIAS[:, 0:1])
                exp_rng = () if "exp" in ablate else (0, 1)
                for i in exp_rng:
                    nc.scalar.activation(EX[:, i * PS:(i + 1) * PS],
                                         W[:, 2 * i * PS:(2 * i + 1) * PS],
                                         AF.Exp, bias=BIAS[:, i:i + 1])
                for i in exp_rng:
                    nc.scalar.activation(EX[:, (2 + i) * PS:(3 + i) * PS],
                                         W[:, (2 * i + 1) * PS:(2 * i + 2) * PS],
                                         AF.Exp, bias=BIAS[:, 2 + i:3 + i])
                for i in exp_rng:
                    nc.scalar.activation(EX[:, (4 + i) * PS:(5 + i) * PS],
                                         W[:, (2 * i + 1) * PS:(2 * i + 2) * PS],
                                         AF.Exp, bias=BIAS[:, 4 + i:5 + i])

                # ---- pair matrices: planes (i,j), plane q = 2i+j ----
                if "tree" in ablate:
                    continue
                nc.vector.tensor_tensor(
                    out=_ap(PM, 0, [[2 * PS, 2], [1, PS]]),
                    in0=_ap(EX, 0, [[PS, 2], [1, PS]]),
                    in1=_ap(EX, 2 * PS, [[PS, 2], [1, PS]]), op=OP.add)
                # P_i1 = delta*wh_i + wd_i (TS into planes {1,3}, then
                # in-place add of wd)
                nc.vector.tensor_scalar(
                    out=_ap(PM, PS, [[2 * PS, 2], [1, PS]]),
                    in0=_ap(EX, 0, [[PS, 2], [1, PS]]),
                    scalar1=c["delta"], scalar2=None, op0=OP.mult)
                nc.vector.tensor_tensor(
                    out=_ap(PM, PS, [[2 * PS, 2], [1, PS]]),
                    in0=_ap(PM, PS, [[2 * PS, 2], [1, PS]]),
                    in1=_ap(EX, 4 * PS, [[PS, 2], [1, PS]]), op=OP.add)

                # ---- pair-0 patch: alpha-init leaf (j-independent) ----
                T1 = wk.tile([P, 2], BF16, tag="T1")
                nc.vector.tensor_scalar(out=T1,
                                        in0=_ap(EX, 0, [[PS, 2], [1, 1]]),
                                        scalar1=c["p1"], scalar2=None,
                                        op0=OP.mult)
                T2 = wk.tile([P, 2], BF16, tag="T2")
                nc.gpsimd.tensor_tensor(out=T2, in0=T1,
                                        in1=_ap(EX, 2 * PS, [[PS, 2], [1, 1]]),
                                        op=OP.add)
                nc.vector.tensor_scalar(
                    out=_ap(PM, 0, [[2 * PS, 2], [PS, 2]]),
                    in0=_ap(T2, 0, [[1, 2], [0, 2]]),
                    scalar1=c["p2"], scalar2=None, op0=OP.mult)

                # ---- linear tree: 6 levels of 2x2 products ----
                # global bit-reversed storage: every level combines the two
                # contiguous halves of each plane (A = second half = odd
                # children = left factor).
                cur, pl = PM, PS
                for v in range(1, 7):
                    h = pl // 2
                    eng = nc.vector if v <= 3 else nc.gpsimd
                    M1 = wk.tile([P, 4 * h], BF16, tag=f"M1_{v}")
                    M2 = wk.tile([P, 4 * h], BF16, tag=f"M2_{v}")
                    NX = wk.tile([P, 4 * h], BF16, tag=f"NX_{v}")
                    oap = [[2 * h, 2], [h, 2], [1, h]]
                    eng.tensor_tensor(
                        out=_ap(M1, 0, oap),
                        in0=_ap(cur, 0 * pl + h, [[2 * pl, 2], [0, 2], [1, h]]),
                        in1=_ap(cur, 0 * pl + 0, [[0, 2], [pl, 2], [1, h]]),
                        op=OP.mult)
                    eng.tensor_tensor(
                        out=_ap(M2, 0, oap),
                        in0=_ap(cur, 1 * pl + h, [[2 * pl, 2], [0, 2], [1, h]]),
                        in1=_ap(cur, 2 * pl + 0, [[0, 2], [pl, 2], [1, h]]),
                        op=OP.mult)
                    eng.tensor_tensor(out=NX, in0=M1, in1=M2, op=OP.add)
                    cur, pl = NX, h

                # ---- Ln -> f32 span logs into LBA[(i,j)][s*G + g] ----
                nc.scalar.activation(
                    _ap(LBA, g, [[G * NBLK, 4], [G, NBLK]]),
                    _ap(cur, 0, [[NBLK, 4], [1, NBLK]]), AF.Ln)

            # ---- top log tree, batched across groups (g interleaved) ----
            if "tree" in ablate or "top" in ablate:
                nc.sync.dma_start(
                    out=bass.AP(tensor=outp[:].tensor, offset=0,
                                ap=[[4, P], [4 * P, G], [1, 4]]),
                    in_=_ap(ACA, 0, [[4, G], [1, 4]]))
                continue
            tpl_s = NBLK
            n_s = NBLK
            src = LBA
            while n_s > 1:
                h = n_s // 2
                S0 = wk.tile([P, 4 * G * h], F32, tag=f"S0_{n_s}")
                S1 = wk.tile([P, 4 * G * h], F32, tag=f"S1_{n_s}")
                MN = wk.tile([P, 4 * G * h], F32, tag=f"MN_{n_s}")
                oap = [[2 * G * h, 2], [G * h, 2], [1, G * h]]

                def a_ap(mu):
                    return _ap(src, mu * G * tpl_s + G * h,
                               [[2 * G * tpl_s, 2], [0, 2], [1, G * h]])

                def b_ap(mu):
                    return _ap(src, 2 * mu * G * tpl_s + 0,
                               [[0, 2], [G * tpl_s, 2], [1, G * h]])

                nc.gpsimd.tensor_tensor(out=_ap(S0, 0, oap), in0=a_ap(0),
                                        in1=b_ap(0), op=OP.add)
                nc.gpsimd.tensor_tensor(out=_ap(S1, 0, oap), in0=a_ap(1),
                                        in1=b_ap(1), op=OP.add)
                nc.vector.tensor_tensor(out=MN, in0=S0, in1=S1, op=OP.min)
                nc.vector.tensor_tensor(out=S0, in0=S0, in1=S1, op=OP.max)
                nc.gpsimd.tensor_tensor(out=MN, in0=MN, in1=S0, op=OP.subtract)
                nc.scalar.activation(MN, MN, AF.Exp)
                nc.scalar.activation(MN, MN, AF.Ln, bias=1.0)
                nc.gpsimd.tensor_tensor(out=S1, in0=S0, in1=MN, op=OP.add)
                src, tpl_s, n_s = S1, h, h

            # final: lT_i0 for each g -> ACA[:, 4g + i]
            nc.vector.tensor_scalar(
                out=_ap(ACA, 0, [[4, G], [1, 2]]),
                in0=_ap(src, 0, [[1, G], [2 * G, 2]]),
                scalar1=1.0, scalar2=None, op0=OP.mult)
            nc.sync.dma_start(
                out=bass.AP(tensor=outp[:].tensor, offset=0,
                            ap=[[4, P], [4 * P, G], [1, 4]]),
                in_=_ap(ACA, 0, [[4, G], [1, 4]]))

    return _split_multiwaits(nc)


_CACHE = {}
_IDX = None


def _indices():
    global _IDX
    if _IDX is None:
        nb = 11                      # log2(PS): global bit-reversal of pairs
        ks = np.zeros(PS, np.int64)
        for i in range(PS):
            b = 0
            for k in range(nb):
                if i >> k & 1:
                    b |= 1 << (nb - 1 - k)
            ks[i] = b
        _IDX = (2 * ks + 1, 2 * ks)     # hi, lo position indices [2048]
    return _IDX


def _get_nc(key, consts, repeat=1):
    if key not in _CACHE:
        _CACHE[key] = _build(consts, repeat=repeat)
    return _CACHE[key]


def _np_crf_fallback(emissions, tags, mask, transitions, start_transitions,
                     end_transitions):
    em = np.asarray(emissions, np.float64)
    tgn = np.asarray(tags, np.int64)
    mk = np.asarray(mask, bool)
    tr = np.asarray(transitions, np.float64)
    st = np.asarray(start_transitions, np.float64)
    en = np.asarray(end_transitions, np.float64)
    B, Ln, T = em.shape
    score = st[tgn[:, 0]] + em[np.arange(B), 0, tgn[:, 0]]
    for l in range(1, Ln):
        emit = em[np.arange(B), l, tgn[:, l]]
        trans = tr[tgn[:, l], tgn[:, l - 1]]
        score += (emit + trans) * mk[:, l]
    alpha = st[None, :] + em[:, 0]
    for l in range(1, Ln):
        sc = alpha[:, None, :] + tr[None, :, :]
        m = sc.max(axis=2, keepdims=True)
        a_new = np.log(np.exp(sc - m).sum(axis=2)) + m[:, :, 0] + em[:, l]
        alpha = np.where(mk[:, l, None], a_new, alpha)
    m = (alpha + en).max(axis=1, keepdims=True)
    logz = np.log(np.exp(alpha + en - m).sum(axis=1)) + m[:, 0]
    sl = np.maximum(mk.sum(axis=1), 1.0)
    return np.float32(((logz - score) / sl).mean())


def kernel(emissions, tags, mask, transitions, start_transitions,
           end_transitions):
    B, Ln, T = emissions.shape
    if not (T == 2 and Ln == L and B == N_CORES * G * P and np.all(mask)):
        return _np_crf_fallback(emissions, tags, mask, transitions,
                                start_transitions, end_transitions)

    tr = np.asarray(transitions, np.float64)
    st = np.asarray(start_transitions, np.float64)
    en = np.asarray(end_transitions, np.float64)
    CB = 0.9 + float(tr.mean())
    consts = _host_consts(tr, CB)
    nc = _get_nc(consts, consts)

    idx_hi, idx_lo = _indices()
    emf = np.asarray(emissions, np.float32)
    tgn = np.asarray(tags, np.int64)

    # host tag-only gold part (bilinear form of transition sum) + en[t_last]
    a = tgn[:, 1:]
    b = tgn[:, :-1]
    sa = a.sum(1, dtype=np.int64)
    sb = b.sum(1, dtype=np.int64)
    sab = (a * b).sum(1, dtype=np.int64)
    cC = tr[1, 1] - tr[1, 0] - tr[0, 1] + tr[0, 0]
    gtag = (tr[0, 0] * (Ln - 1) + (tr[1, 0] - tr[0, 0]) * sa
            + (tr[0, 1] - tr[0, 0]) * sb + cC * sab + en[tgn[:, -1]])

    BS = G * P
    in_maps = []
    for cidx in range(N_CORES):
        esl = np.array(emf[cidx * BS:(cidx + 1) * BS])     # [BS, L, 2]
        esl[:, 0, :] += st[None, :].astype(np.float32)
        tsl = tgn[cidx * BS:(cidx + 1) * BS]
        EMp = np.empty((BS, 4, PS), NPBF)
        EMp[:, 0] = esl[:, idx_hi, 0]
        EMp[:, 1] = esl[:, idx_hi, 1]
        EMp[:, 2] = esl[:, idx_lo, 0]
        EMp[:, 3] = esl[:, idx_lo, 1]
        TGp = np.empty((BS, 2, PS), NPBF)
        TGp[:, 0] = tsl[:, idx_hi]
        TGp[:, 1] = tsl[:, idx_lo]
        in_maps.append({
            "emissions": np.ascontiguousarray(EMp.reshape(BS, 4 * PS)),
            "tagsf": np.ascontiguousarray(TGp.reshape(BS, 2 * PS)),
        })

    res = run_bass_kernel_spmd(nc, in_maps, core_ids=list(range(N_CORES)))
    outs = np.concatenate([r["outp"] for r in res.results])   # [B, 4]
    lt0 = outs[:, 0].astype(np.float64)
    lt1 = outs[:, 1].astype(np.float64)
    se0 = outs[:, 2].astype(np.float64)
    stD = outs[:, 3].astype(np.float64)
    a0 = en[0] + lt0
    a1 = en[1] + lt1
    mx = np.maximum(a0, a1)
    logZ = CB * Ln + mx + np.log1p(np.exp(np.minimum(a0, a1) - mx))
    gold = se0 + stD + gtag
    nll = (logZ - gold) / Ln
    return np.float32(nll.mean())
